# revision 1
# baseline (speedup 1.0000x reference)
"""Trainium2 Bass kernel for nn_MixtralOfExpertsLayer (MoE, top-2 of 8 experts).

Sharding: token-parallel over 8 NeuronCores. Each core owns 1024 tokens
end-to-end (router + all-expert FFN + weighted combine), so no collectives
are needed; the host only splits x and concatenates the per-core outputs.

Per-core pipeline (T-formulation: activations kept as [feature, token]):
  - gate logits in exact fp32 on the PE, top-2 via vector max/max_index,
    renormalized weights via the sigmoid identity g2 = sigmoid(l2-l1).
  - dense FFN over all 8 experts in float32r (full-rate PE), scaled by the
    masked gate weights, accumulated in SBUF.
  - PE-transpose back to [token, feature] and DMA out.
"""

import sys

import numpy as np

sys.path.insert(0, "/opt/trn_rl_repo")

from concourse import bacc, bass, mybir  # noqa: E402
import concourse.tile as tile  # noqa: E402
from concourse.bass_utils import run_bass_kernel_spmd  # noqa: E402
from concourse.masks import make_identity  # noqa: E402

B, T, D, H, O, E = 4, 2048, 1024, 2048, 1024, 8
N_CORES = 8
NTOK = (B * T) // N_CORES  # 1024 tokens per core
P = 128
KD = D // P   # 8 contraction tiles for D
MH = H // P   # 16 partition tiles for H
MO = O // P   # 8 partition tiles for O
TM = NTOK // P  # 8 token tiles per core
NCH = 512     # matmul moving free-dim (one PSUM bank in fp32)
NNC = NTOK // NCH  # 2

f32 = mybir.dt.float32
f32r = mybir.dt.float32r
u32 = mybir.dt.uint32
AF = mybir.ActivationFunctionType
ALU = mybir.AluOpType

_CACHE: dict = {}


def _build():
    nc = bacc.Bacc("TRN2", target_bir_lowering=False, debug=False,
                   num_devices=N_CORES)
    xt = nc.declare_dram_parameter("xt", [D, NTOK], f32r, isOutput=False)
    xtg = nc.declare_dram_parameter("xtg", [D, NTOK], f32, isOutput=False)
    wg = nc.declare_dram_parameter("wg", [D, E], f32, isOutput=False)
    bgb = nc.declare_dram_parameter("bgb", [P, E], f32, isOutput=False)
    w1 = nc.declare_dram_parameter("w1", [E, D, H], f32r, isOutput=False)
    b1 = nc.declare_dram_parameter("b1", [E, H, 1], f32, isOutput=False)
    w2 = nc.declare_dram_parameter("w2", [E, H, O], f32r, isOutput=False)
    b2 = nc.declare_dram_parameter("b2", [O, 1], f32, isOutput=False)
    y = nc.declare_dram_parameter("y", [NTOK, O], f32, isOutput=True)

    with tile.TileContext(nc) as tc:
        with (
            tc.tile_pool(name="const", bufs=1) as constp,
            tc.tile_pool(name="res", bufs=1) as resp,
            tc.tile_pool(name="wstr", bufs=3) as wp,
            tc.tile_pool(name="gate", bufs=2) as gp,
            tc.tile_pool(name="tmp", bufs=3) as tmpp,
            tc.tile_pool(name="outs", bufs=2) as outp,
            tc.tile_pool(name="psmm", bufs=4, space="PSUM") as psmm,
            tc.tile_pool(name="psg", bufs=1, space="PSUM") as psg,
            tc.tile_pool(name="pstr", bufs=2, space="PSUM") as pstr,
        ):
            # ---- constants ----
            idn = constp.tile([P, P], f32, tag="idn")
            make_identity(nc, idn[:])
            iot = constp.tile([P, E], f32, tag="iot")
            nc.gpsimd.iota(iot[:], pattern=[[1, E]], base=0,
                           channel_multiplier=0,
                           allow_small_or_imprecise_dtypes=True)
            bgsb = constp.tile([P, E], f32, tag="bgsb")
            nc.sync.dma_start(out=bgsb[:], in_=bgb[:])
            b2sb = constp.tile([P, MO], f32, tag="b2sb")
            nc.sync.dma_start(
                out=b2sb[:],
                in_=b2.rearrange("(om p) one -> p (om one)", p=P))
            wgsb = constp.tile([P, KD * E], f32, tag="wgsb")
            nc.sync.dma_start(
                out=wgsb[:].rearrange("p (kd e) -> p kd e", e=E),
                in_=wg.rearrange("(kd p) e -> p kd e", p=P))

            # ---- resident activations: x^T in f32r for the FFN ----
            xtr = []
            for kd in range(KD):
                t = resp.tile([P, NTOK], f32r, tag=f"xtr{kd}", name=f"xtr{kd}")
                nc.sync.dma_start(out=t[:], in_=xt[kd * P:(kd + 1) * P, :])
                xtr.append(t)

            # ---- gate: logits, top-2, renormalized weights ----
            # gtrow[e][0, tok]: per-expert gate weight row (0 if not routed)
            gtrow = resp.tile([1, E * NTOK], f32, tag="gtrow",
                              name="gtrow")
            for tm in range(TM):
                ts = slice(tm * P, (tm + 1) * P)
                pg = psg.tile([P, E], f32, tag="pg")
                for kd in range(KD):
                    xg = gp.tile([P, P], f32, tag="xg")
                    nc.sync.dma_start(
                        out=xg[:], in_=xtg[kd * P:(kd + 1) * P, ts])
                    nc.tensor.matmul(
                        pg[:], lhsT=xg[:],
                        rhs=wgsb[:, kd * E:(kd + 1) * E],
                        start=(kd == 0), stop=(kd == KD - 1))
                lg = gp.tile([P, E], f32, tag="lg")
                nc.vector.tensor_add(out=lg[:], in0=pg[:], in1=bgsb[:])
                vm = gp.tile([P, E], f32, tag="vm")
                nc.vector.max(vm[:], lg[:])
                vi = gp.tile([P, E], u32, tag="vi")
                nc.vector.max_index(vi[:], vm[:], lg[:])
                vif = gp.tile([P, E], f32, tag="vif")
                nc.vector.tensor_copy(out=vif[:], in_=vi[:])
                dlt = gp.tile([P, 1], f32, tag="dlt")
                nc.vector.tensor_sub(dlt[:], vm[:, 1:2], vm[:, 0:1])
                g2 = gp.tile([P, 1], f32, tag="g2")
                nc.scalar.activation(out=g2[:], in_=dlt[:], func=AF.Sigmoid)
                g1 = gp.tile([P, 1], f32, tag="g1")
                nc.vector.tensor_scalar(g1[:], g2[:], -1.0, 1.0,
                                        ALU.mult, ALU.add)
                m1 = gp.tile([P, E], f32, tag="m1")
                nc.vector.tensor_tensor(
                    out=m1[:], in0=vif[:, 0:1].to_broadcast([P, E]),
                    in1=iot[:], op=ALU.is_equal)
                m2 = gp.tile([P, E], f32, tag="m2")
                nc.vector.tensor_tensor(
                    out=m2[:], in0=vif[:, 1:2].to_broadcast([P, E]),
                    in1=iot[:], op=ALU.is_equal)
                t1 = gp.tile([P, E], f32, tag="t1")
                nc.vector.tensor_tensor(
                    out=t1[:], in0=m1[:], in1=g1[:].to_broadcast([P, E]),
                    op=ALU.mult)
                t2 = gp.tile([P, E], f32, tag="t2")
                nc.vector.tensor_tensor(
                    out=t2[:], in0=m2[:], in1=g2[:].to_broadcast([P, E]),
                    op=ALU.mult)
                gv = gp.tile([P, E], f32, tag="gv")
                nc.vector.tensor_add(out=gv[:], in0=t1[:], in1=t2[:])
                for e in range(E):
                    pt1 = pstr.tile([1, P], f32, tag="tr", name="pt1")
                    nc.tensor.transpose(out=pt1[:], in_=gv[:, e:e + 1],
                                        identity=idn[:])
                    nc.vector.tensor_copy(
                        out=gtrow[:, e * NTOK + tm * P:e * NTOK + (tm + 1) * P],
                        in_=pt1[:])

            # ---- dense FFN over experts, f32r, gate-scaled accumulate ----
            acc = [resp.tile([P, NTOK], f32, tag=f"acc{om}", name=f"acc{om}")
                   for om in range(MO)]
            ht = [resp.tile([P, NTOK], f32r, tag=f"ht{hm}", name=f"ht{hm}")
                  for hm in range(MH)]
            for e in range(E):
                gtb = tmpp.tile([P, NTOK], f32, tag="gtb", name="gtb", bufs=2)
                nc.gpsimd.partition_broadcast(
                    gtb[:], gtrow[:, e * NTOK:(e + 1) * NTOK])
                for hm in range(MH):
                    w1sb = wp.tile([P, KD * P], f32r, tag="w1sb", bufs=2)
                    nc.sync.dma_start(
                        out=w1sb[:].rearrange("p (kd h) -> p kd h", h=P),
                        in_=w1[e, :, hm * P:(hm + 1) * P]
                        .rearrange("(kd p) h -> p kd h", p=P))
                    b1c = tmpp.tile([P, 1], f32, tag="b1c")
                    nc.sync.dma_start(
                        out=b1c[:], in_=b1[e, hm * P:(hm + 1) * P, :])
                    for nn in range(NNC):
                        ns = slice(nn * NCH, (nn + 1) * NCH)
                        ph = psmm.tile([P, NCH], f32, tag="mm")
                        for kd in range(KD):
                            nc.tensor.matmul(
                                ph[:], lhsT=w1sb[:, kd * P:(kd + 1) * P],
                                rhs=xtr[kd][:, ns],
                                start=(kd == 0), stop=(kd == KD - 1))
                        nc.scalar.activation(
                            out=ht[hm][:, ns], in_=ph[:], func=AF.Relu,
                            bias=b1c[:])
                for om in range(MO):
                    w2sb = wp.tile([P, MH * P], f32r, tag="w2sb", bufs=2)
                    nc.sync.dma_start(
                        out=w2sb[:].rearrange("p (kh o) -> p kh o", o=P),
                        in_=w2[e, :, om * P:(om + 1) * P]
                        .rearrange("(kh p) o -> p kh o", p=P))
                    for nn in range(NNC):
                        ns = slice(nn * NCH, (nn + 1) * NCH)
                        po = psmm.tile([P, NCH], f32, tag="mm")
                        for kh in range(MH):
                            nc.tensor.matmul(
                                po[:], lhsT=w2sb[:, kh * P:(kh + 1) * P],
                                rhs=ht[kh][:, ns],
                                start=(kh == 0), stop=(kh == MH - 1))
                        grow = gtb[:, ns]
                        if e == 0:
                            nc.vector.tensor_tensor(
                                out=acc[om][:, ns], in0=po[:], in1=grow,
                                op=ALU.mult)
                        else:
                            tmp = tmpp.tile([P, NCH], f32, tag="sc", bufs=2)
                            nc.vector.tensor_tensor(
                                out=tmp[:], in0=po[:], in1=grow, op=ALU.mult)
                            nc.vector.tensor_add(
                                out=acc[om][:, ns], in0=acc[om][:, ns],
                                in1=tmp[:])

            # ---- bias2, transpose back to [token, feature], store ----
            for om in range(MO):
                nc.vector.tensor_tensor(
                    out=acc[om][:], in0=acc[om][:],
                    in1=b2sb[:, om:om + 1].to_broadcast([P, NTOK]),
                    op=ALU.add)
            for tm in range(TM):
                osb = outp.tile([P, O], f32, tag="osb", bufs=1)
                for om in range(MO):
                    ptt = pstr.tile([P, P], f32, tag="tr", name="ptt")
                    nc.tensor.transpose(
                        out=ptt[:], in_=acc[om][:, tm * P:(tm + 1) * P],
                        identity=idn[:])
                    nc.vector.tensor_copy(
                        out=osb[:, om * P:(om + 1) * P], in_=ptt[:])
                nc.sync.dma_start(
                    out=y[tm * P:(tm + 1) * P, :], in_=osb[:])

    nc.compile()
    return nc


def kernel(x, num_experts_chosen, W_gate, b_gate, W1, b1, W2, b2):
    assert int(num_experts_chosen) == 2
    x = np.ascontiguousarray(np.asarray(x, dtype=np.float32))
    W_gate = np.ascontiguousarray(np.asarray(W_gate, dtype=np.float32))
    b_gate = np.asarray(b_gate, dtype=np.float32)
    W1 = np.ascontiguousarray(np.asarray(W1, dtype=np.float32))
    b1 = np.asarray(b1, dtype=np.float32)
    W2 = np.ascontiguousarray(np.asarray(W2, dtype=np.float32))
    b2 = np.asarray(b2, dtype=np.float32)

    if "nc" not in _CACHE:
        _CACHE["nc"] = _build()
    nc = _CACHE["nc"]

    xtok = x.reshape(B * T, D)
    bgb = np.ascontiguousarray(np.broadcast_to(b_gate[None, :], (P, E)))
    b1c = np.ascontiguousarray(b1[:, :, None])
    b2c = np.ascontiguousarray(b2[:, None])
    in_maps = []
    for c in range(N_CORES):
        xs = np.ascontiguousarray(xtok[c * NTOK:(c + 1) * NTOK, :].T)
        in_maps.append({
            "xt": xs, "xtg": xs, "wg": W_gate, "bgb": bgb,
            "w1": W1, "b1": b1c, "w2": W2, "b2": b2c,
        })
    res = run_bass_kernel_spmd(nc, in_maps, core_ids=list(range(N_CORES)))
    out = np.concatenate([res.results[c]["y"] for c in range(N_CORES)], axis=0)
    return out.reshape(B, T, O)



# revision 2
# speedup vs baseline: 8.8463x; 8.8463x over previous
"""Trainium2 Bass kernel for nn_MixtralOfExpertsLayer (MoE, top-2 of 8 experts).

Sharding: expert-parallel with routed dispatch. The router (0.3% of the
FLOPs) runs on host in fp64; each of the 8 NeuronCores owns exactly one
expert and receives ONLY that expert's weights (bf16) plus the tokens
routed to it (bf16, capacity-padded). This sends ~14 MB/core instead of
~137 MB/core (weights for all 8 experts + replicated x) and computes the
top-2 FFN (2/8 of the dense work) instead of all experts.

Per-core device pipeline, transpose-free:
  h[H-part, tok]  = relu(W1^T x + b1)   (x kept feature-major [D, tok])
  y[tok-part, O]  = (h^T W2) * g_tok    (W2 matmul emits token-major)
All matmuls bf16 with fp32 PSUM accumulation, full PE rate.

The capacity (CAP tokens/expert) is static; expected load is 2048 +- 39,
CAP=2560 is ~13 sigma above the mean. If an expert ever exceeds CAP the
host runs additional rounds with the same NEFF, so correctness never
depends on the capacity.
"""

import sys

import numpy as np

sys.path.insert(0, "/opt/trn_rl_repo")

import ml_dtypes  # noqa: E402

from concourse import bacc, mybir  # noqa: E402
import concourse.tile as tile  # noqa: E402
from concourse.bass_utils import run_bass_kernel_spmd  # noqa: E402

B, T, D, H, O, E = 4, 2048, 1024, 2048, 1024, 8
N_CORES = 8
P = 128
CAP = 2560          # per-expert token capacity (multiple of NCH)
KD = D // P         # 8 contraction tiles over D
MH = H // P         # 16 partition tiles over H
NCH = 512           # token chunk = one PSUM bank in fp32
NCHUNK = CAP // NCH  # 5
TT = NCH // P       # 4 token tiles per chunk
TTOT = CAP // P     # 20 token tiles total
OC = 512            # output free-dim chunk
NOC = O // OC       # 2

f32 = mybir.dt.float32
bf16 = mybir.dt.bfloat16
nbf16 = ml_dtypes.bfloat16
AF = mybir.ActivationFunctionType
ALU = mybir.AluOpType

_CACHE: dict = {}


def _build():
    nc = bacc.Bacc("TRN2", target_bir_lowering=False, debug=False,
                   num_devices=N_CORES)
    xg = nc.declare_dram_parameter("xg", [D, CAP], bf16, isOutput=False)
    w1 = nc.declare_dram_parameter("w1", [D, H], bf16, isOutput=False)
    w2 = nc.declare_dram_parameter("w2", [H, O], bf16, isOutput=False)
    b1 = nc.declare_dram_parameter("b1", [H, 1], f32, isOutput=False)
    gt = nc.declare_dram_parameter("gt", [CAP, 1], f32, isOutput=False)
    y = nc.declare_dram_parameter("y", [CAP, O], bf16, isOutput=True)

    with tile.TileContext(nc) as tc:
        with (
            tc.tile_pool(name="const", bufs=1) as constp,
            tc.tile_pool(name="xres", bufs=1) as xp,
            tc.tile_pool(name="wres", bufs=1) as wp,
            tc.tile_pool(name="hbuf", bufs=2) as hp,
            tc.tile_pool(name="ybuf", bufs=3) as yp,
            tc.tile_pool(name="ps1", bufs=4, space="PSUM") as ps1,
            tc.tile_pool(name="ps2", bufs=4, space="PSUM") as ps2,
        ):
            # ---- constants: biases and gate weights, one DMA each ----
            b1sb = constp.tile([P, MH], f32, tag="b1sb")
            nc.sync.dma_start(
                out=b1sb[:],
                in_=b1.rearrange("(hm p) one -> p (hm one)", p=P))
            gtsb = constp.tile([P, TTOT], f32, tag="gtsb")
            nc.sync.dma_start(
                out=gtsb[:],
                in_=gt.rearrange("(tt p) one -> p (tt one)", p=P))

            # ---- resident inputs: x^T, W1, W2 (all bf16) ----
            xsb = []
            for kd in range(KD):
                t = xp.tile([P, CAP], bf16, tag=f"x{kd}")
                nc.sync.dma_start(out=t[:], in_=xg[kd * P:(kd + 1) * P, :])
                xsb.append(t)
            w1sb = []
            for kd in range(KD):
                t = wp.tile([P, H], bf16, tag=f"w1_{kd}")
                nc.sync.dma_start(out=t[:], in_=w1[kd * P:(kd + 1) * P, :])
                w1sb.append(t)
            w2sb = []
            for kh in range(MH):
                t = wp.tile([P, O], bf16, tag=f"w2_{kh}")
                nc.sync.dma_start(out=t[:], in_=w2[kh * P:(kh + 1) * P, :])
                w2sb.append(t)

            # ---- FFN over token chunks ----
            for c in range(NCHUNK):
                cs = slice(c * NCH, (c + 1) * NCH)
                hts = []
                for hm in range(MH):
                    ph = ps1.tile([P, NCH], f32, tag="ph")
                    for kd in range(KD):
                        nc.tensor.matmul(
                            ph[:], lhsT=w1sb[kd][:, hm * P:(hm + 1) * P],
                            rhs=xsb[kd][:, cs],
                            start=(kd == 0), stop=(kd == KD - 1))
                    ht = hp.tile([P, NCH], bf16, tag=f"h{hm}")
                    nc.scalar.activation(out=ht[:], in_=ph[:], func=AF.Relu,
                                         bias=b1sb[:, hm:hm + 1])
                    hts.append(ht)
                for tt in range(TT):
                    tglob = c * TT + tt
                    ysb = yp.tile([P, O], bf16, tag="ysb")
                    for oc in range(NOC):
                        po = ps2.tile([P, OC], f32, tag="po")
                        for kh in range(MH):
                            nc.tensor.matmul(
                                po[:], lhsT=hts[kh][:, tt * P:(tt + 1) * P],
                                rhs=w2sb[kh][:, oc * OC:(oc + 1) * OC],
                                start=(kh == 0), stop=(kh == MH - 1))
                        nc.vector.tensor_tensor(
                            out=ysb[:, oc * OC:(oc + 1) * OC], in0=po[:],
                            in1=gtsb[:, tglob:tglob + 1].to_broadcast([P, OC]),
                            op=ALU.mult)
                    nc.sync.dma_start(
                        out=y[tglob * P:(tglob + 1) * P, :], in_=ysb[:])

    nc.compile()
    return nc


def kernel(x, num_experts_chosen, W_gate, b_gate, W1, b1, W2, b2):
    assert int(num_experts_chosen) == 2
    x2d = np.asarray(x, np.float32).reshape(B * T, D)
    Wg = np.asarray(W_gate, np.float64)
    bg = np.asarray(b_gate, np.float64)
    W1 = np.asarray(W1, np.float32)
    b1 = np.asarray(b1, np.float32)
    W2 = np.asarray(W2, np.float32)
    b2 = np.asarray(b2, np.float32)

    # ---- router on host: softmax over experts, top-2, L1 renormalize ----
    logits = x2d.astype(np.float64) @ Wg + bg
    order = np.argsort(-logits, axis=-1, kind="stable")  # ties: lower index
    top2 = order[:, :2]
    mx = logits.max(-1, keepdims=True)
    pexp = np.exp(logits - mx)
    gating = pexp / pexp.sum(-1, keepdims=True)
    pv = np.take_along_axis(gating, top2, 1)
    g = (pv / np.maximum(pv.sum(1, keepdims=True), 1e-12)).astype(np.float32)

    xbf = x2d.astype(nbf16)
    W1b = W1.astype(nbf16)
    W2b = W2.astype(nbf16)

    idx_e, g_e = [], []
    for e in range(E):
        s0 = top2[:, 0] == e
        s1 = top2[:, 1] == e
        idx = np.nonzero(s0 | s1)[0]
        ge = np.where(s0[idx], g[idx, 0], g[idx, 1]).astype(np.float32)
        idx_e.append(idx)
        g_e.append(ge)

    if "nc" not in _CACHE:
        _CACHE["nc"] = _build()
    nc = _CACHE["nc"]

    out2d = np.zeros((B * T, O), np.float32)
    maxn = max(len(i) for i in idx_e)
    rounds = max(1, -(-maxn // CAP))
    for r in range(rounds):
        in_maps = []
        for e in range(E):
            sl = idx_e[e][r * CAP:(r + 1) * CAP]
            n = len(sl)
            xgb = np.zeros((D, CAP), nbf16)
            gtb = np.zeros((CAP, 1), np.float32)
            if n:
                xgb[:, :n] = xbf[sl].T
                gtb[:n, 0] = g_e[e][r * CAP:r * CAP + n]
            in_maps.append({
                "xg": xgb, "w1": W1b[e], "w2": W2b[e],
                "b1": np.ascontiguousarray(b1[e][:, None]), "gt": gtb,
            })
        res = run_bass_kernel_spmd(nc, in_maps, core_ids=list(range(N_CORES)))
        for e in range(E):
            sl = idx_e[e][r * CAP:(r + 1) * CAP]
            if len(sl):
                out2d[sl] += res.results[e]["y"][:len(sl)].astype(np.float32)

    if b2.any():
        out2d += g[:, 0, None] * b2[top2[:, 0]] \
            + g[:, 1, None] * b2[top2[:, 1]]
    return out2d.reshape(B, T, O)


# revision 4
# speedup vs baseline: 12.2577x; 1.3856x over previous
"""Trainium2 Bass kernel for nn_MixtralOfExpertsLayer (MoE, top-2 of 8 experts).

Sharding: expert-parallel with routed dispatch. The router (0.3% of the
FLOPs) runs on host in fp64; each of the 8 NeuronCores owns exactly one
expert and receives ONLY that expert's weights (bf16) plus the tokens
routed to it (bf16, capacity-padded). This sends ~14 MB/core instead of
~137 MB/core (weights for all 8 experts + replicated x) and computes the
top-2 FFN (2/8 of the dense work) instead of all experts.

Per-core device pipeline, transpose-free:
  h[H-part, tok]  = relu(W1^T x + b1)   (x kept feature-major [D, tok])
  y[tok-part, O]  = (h^T W2) * g_tok    (W2 matmul emits token-major)
then a software-DGE scatter-add places each token's scaled expert output
into a local [BT, O] accumulator at its global position, and a
ReduceScatter over the 8 cores performs the top-2 combine on device, so
each core returns only its [BT/8, O] shard (4x less output traffic than
returning per-expert outputs).

The capacity (CAP tokens/expert) is static; expected load is 2048 +- 39,
CAP=2560 is ~13 sigma above the mean. If an expert ever exceeds CAP the
host runs additional rounds with the same NEFF, so correctness never
depends on the capacity. Capacity-padding tokens carry gate weight 0 and
scatter to row 0 as exact +-0 no-op adds.
"""

import sys

import numpy as np

sys.path.insert(0, "/opt/trn_rl_repo")

import ml_dtypes  # noqa: E402

from concourse import bacc, mybir  # noqa: E402
import concourse.tile as tile  # noqa: E402
from concourse.bass_utils import run_bass_kernel_spmd  # noqa: E402

B, T, D, H, O, E = 4, 2048, 1024, 2048, 1024, 8
BT = B * T
N_CORES = 8
SHARD = BT // N_CORES  # 1024 output rows per core
P = 128
CAP = 2560          # per-expert token capacity (multiple of NCH)
KD = D // P         # 8 contraction tiles over D
MH = H // P         # 16 partition tiles over H
NCH = 512           # token chunk = one PSUM bank in fp32
NCHUNK = CAP // NCH  # 5
TT = NCH // P       # 4 token tiles per chunk
TTOT = CAP // P     # 20 token tiles total
OC = 512            # output free-dim chunk
NOC = O // OC       # 2
NIDX = CAP // 16    # scatter index columns

f32 = mybir.dt.float32
bf16 = mybir.dt.bfloat16
i16 = mybir.dt.int16
nbf16 = ml_dtypes.bfloat16
AF = mybir.ActivationFunctionType
ALU = mybir.AluOpType

_CACHE: dict = {}


def _build():
    nc = bacc.Bacc("TRN2", target_bir_lowering=False, debug=False,
                   num_devices=N_CORES)
    xg = nc.declare_dram_parameter("xg", [D, CAP], bf16, isOutput=False)
    w1 = nc.declare_dram_parameter("w1", [D, H], bf16, isOutput=False)
    w2 = nc.declare_dram_parameter("w2", [H, O], bf16, isOutput=False)
    b1 = nc.declare_dram_parameter("b1", [H, 1], f32, isOutput=False)
    gt = nc.declare_dram_parameter("gt", [CAP, 1], f32, isOutput=False)
    idx = nc.declare_dram_parameter("idx", [P, NIDX], i16, isOutput=False)
    y = nc.declare_dram_parameter("y", [SHARD, O], bf16, isOutput=True)

    with tile.TileContext(nc) as tc:
        with (
            tc.tile_pool(name="const", bufs=1) as constp,
            tc.tile_pool(name="xres", bufs=1) as xp,
            tc.tile_pool(name="wres", bufs=1) as wp,
            tc.tile_pool(name="hbuf", bufs=2) as hp,
            tc.tile_pool(name="yall", bufs=1) as yp,
            tc.tile_pool(name="dram", bufs=1, space="DRAM") as dramp,
            tc.tile_pool(name="ps1", bufs=4, space="PSUM") as ps1,
            tc.tile_pool(name="ps2", bufs=4, space="PSUM") as ps2,
        ):
            # ---- constants: biases, gate weights, scatter indices ----
            b1sb = constp.tile([P, MH], f32, tag="b1sb")
            nc.sync.dma_start(
                out=b1sb[:],
                in_=b1.rearrange("(hm p) one -> p (hm one)", p=P))
            gtsb = constp.tile([P, TTOT], f32, tag="gtsb")
            nc.sync.dma_start(
                out=gtsb[:],
                in_=gt.rearrange("(tt p) one -> p (tt one)", p=P))
            idxsb = constp.tile([P, NIDX], i16, tag="idxsb")
            nc.sync.dma_start(out=idxsb[:], in_=idx[:, :])

            # ---- zero the local combine accumulator [BT, O] ----
            acc = dramp.tile([BT, O], bf16, tag="acc")
            zt = constp.tile([P, O], bf16, tag="zt")
            nc.gpsimd.memset(zt[:], 0.0)
            for r in range(BT // P):
                nc.sync.dma_start(out=acc[r * P:(r + 1) * P, :], in_=zt[:])

            # ---- resident inputs: x^T, W1, W2 (all bf16) ----
            xsb = []
            for kd in range(KD):
                t = xp.tile([P, CAP], bf16, tag=f"x{kd}")
                nc.sync.dma_start(out=t[:], in_=xg[kd * P:(kd + 1) * P, :])
                xsb.append(t)
            w1sb = []
            for kd in range(KD):
                t = wp.tile([P, H], bf16, tag=f"w1_{kd}")
                nc.sync.dma_start(out=t[:], in_=w1[kd * P:(kd + 1) * P, :])
                w1sb.append(t)
            w2sb = []
            for kh in range(MH):
                t = wp.tile([P, O], bf16, tag=f"w2_{kh}")
                nc.sync.dma_start(out=t[:], in_=w2[kh * P:(kh + 1) * P, :])
                w2sb.append(t)

            # ---- FFN over token chunks; scaled outputs land in ysb_all ----
            ysb_all = yp.tile([P, TTOT * O], bf16, tag="ysb")
            for c in range(NCHUNK):
                cs = slice(c * NCH, (c + 1) * NCH)
                hts = []
                for hm in range(MH):
                    ph = ps1.tile([P, NCH], f32, tag="ph")
                    for kd in range(KD):
                        nc.tensor.matmul(
                            ph[:], lhsT=w1sb[kd][:, hm * P:(hm + 1) * P],
                            rhs=xsb[kd][:, cs],
                            start=(kd == 0), stop=(kd == KD - 1))
                    ht = hp.tile([P, NCH], bf16, tag=f"h{hm}")
                    nc.scalar.activation(out=ht[:], in_=ph[:], func=AF.Relu,
                                         bias=b1sb[:, hm:hm + 1])
                    hts.append(ht)
                for tt in range(TT):
                    tglob = c * TT + tt
                    for oc in range(NOC):
                        po = ps2.tile([P, OC], f32, tag="po")
                        for kh in range(MH):
                            nc.tensor.matmul(
                                po[:], lhsT=hts[kh][:, tt * P:(tt + 1) * P],
                                rhs=w2sb[kh][:, oc * OC:(oc + 1) * OC],
                                start=(kh == 0), stop=(kh == MH - 1))
                        nc.vector.tensor_tensor(
                            out=ysb_all[:, tglob * O + oc * OC:
                                        tglob * O + (oc + 1) * OC],
                            in0=po[:],
                            in1=gtsb[:, tglob:tglob + 1].to_broadcast([P, OC]),
                            op=ALU.mult)

            # ---- scatter into the accumulator, combine across cores ----
            nc.gpsimd.dma_scatter_add(
                acc[:],
                ysb_all[:].rearrange("p (t o) -> p t o", o=O),
                idxsb[:],
                CAP,
                CAP,
                O)
            rsout = dramp.tile([SHARD, O], bf16, tag="rsout")
            nc.gpsimd.collective_compute(
                "ReduceScatter",
                ALU.add,
                replica_groups=[list(range(N_CORES))],
                ins=[acc[:]],
                outs=[rsout[:]])
            nc.sync.dma_start(out=y[:, :], in_=rsout[:])

    nc.compile()
    return nc


def kernel(x, num_experts_chosen, W_gate, b_gate, W1, b1, W2, b2):
    assert int(num_experts_chosen) == 2
    x2d = np.asarray(x, np.float32).reshape(BT, D)
    Wg = np.asarray(W_gate, np.float64)
    bg = np.asarray(b_gate, np.float64)
    W1 = np.asarray(W1, np.float32)
    b1 = np.asarray(b1, np.float32)
    W2 = np.asarray(W2, np.float32)
    b2 = np.asarray(b2, np.float32)

    # ---- router on host: softmax over experts, top-2, L1 renormalize ----
    logits = x2d.astype(np.float64) @ Wg + bg
    order = np.argsort(-logits, axis=-1, kind="stable")  # ties: lower index
    top2 = order[:, :2]
    mx = logits.max(-1, keepdims=True)
    pexp = np.exp(logits - mx)
    gating = pexp / pexp.sum(-1, keepdims=True)
    pv = np.take_along_axis(gating, top2, 1)
    g = (pv / np.maximum(pv.sum(1, keepdims=True), 1e-12)).astype(np.float32)

    xbf = x2d.astype(nbf16)
    W1b = W1.astype(nbf16)
    W2b = W2.astype(nbf16)

    idx_e, g_e = [], []
    for e in range(E):
        s0 = top2[:, 0] == e
        s1 = top2[:, 1] == e
        sel = np.nonzero(s0 | s1)[0]
        ge = np.where(s0[sel], g[sel, 0], g[sel, 1]).astype(np.float32)
        idx_e.append(sel)
        g_e.append(ge)

    if "nc" not in _CACHE:
        _CACHE["nc"] = _build()
    nc = _CACHE["nc"]

    out2d = np.zeros((BT, O), np.float32)
    maxn = max(len(i) for i in idx_e)
    rounds = max(1, -(-maxn // CAP))
    for r in range(rounds):
        in_maps = []
        for e in range(E):
            sl = idx_e[e][r * CAP:(r + 1) * CAP]
            n = len(sl)
            xgb = np.zeros((D, CAP), nbf16)
            gtb = np.zeros((CAP, 1), np.float32)
            dest = np.zeros(CAP, np.int16)  # pad -> row 0, zero payload
            if n:
                xgb[:, :n] = xbf[sl].T
                gtb[:n, 0] = g_e[e][r * CAP:r * CAP + n]
                dest[:n] = sl.astype(np.int16)
            # idx table is read per-16-partition group by the 8 gpsimd
            # cores -> must be replicated into all 8 groups
            idxb = np.tile(dest.reshape(NIDX, 16).T, (8, 1))
            in_maps.append({
                "xg": xgb, "w1": W1b[e], "w2": W2b[e],
                "b1": np.ascontiguousarray(b1[e][:, None]), "gt": gtb,
                "idx": idxb,
            })
        res = run_bass_kernel_spmd(nc, in_maps, core_ids=list(range(N_CORES)))
        for c in range(N_CORES):
            out2d[c * SHARD:(c + 1) * SHARD] += \
                res.results[c]["y"].astype(np.float32)

    if b2.any():
        out2d += g[:, 0, None] * b2[top2[:, 0]] \
            + g[:, 1, None] * b2[top2[:, 1]]
    return out2d.reshape(B, T, O)


# revision 12
# speedup vs baseline: 14.3339x; 1.1694x over previous
"""Trainium2 Bass kernel for nn_MixtralOfExpertsLayer (MoE, top-2 of 8 experts).

Sharding: expert-parallel with routed dispatch. The router (0.3% of the
FLOPs) runs on host in fp64; each of the 8 NeuronCores owns exactly one
expert and receives ONLY that expert's weights (bf16) plus the tokens
routed to it (bf16, capacity-padded). This sends ~14 MB/core instead of
~137 MB/core (weights for all 8 experts + replicated x) and computes the
top-2 FFN (2/8 of the dense work) instead of all experts.

Per-core device pipeline, transpose-free:
  h[H-part, tok]  = relu(W1^T x + b1)   (x kept feature-major [D, tok])
  y[tok-part, O]  = (h^T W2) * g_tok    (W2 matmul emits token-major)
then a software-DGE scatter-add places each token's scaled expert output
into a local [BT, O] accumulator at its global position, and a
ReduceScatter over the 8 cores performs the top-2 combine on device, so
each core returns only its [BT/8, O] shard (4x less output traffic than
returning per-expert outputs).

x is shipped token-sharded ([BT/8, D] per core), AllGathered on device,
and each core picks its expert's tokens with a transposing dma_gather
(producing the feature-major layout the matmuls need directly), so x
costs 17 MB of host->device traffic instead of 42 MB gathered.

The capacity (CAP tokens/expert) is static; expected load is 2048 +- 39,
CAP=2560 is ~13 sigma above the mean. If an expert ever exceeds CAP the
host runs additional rounds with the same NEFF, so correctness never
depends on the capacity. Capacity-padding tokens carry gate weight 0 and
scatter to row 0 as exact +-0 no-op adds.
"""

import sys

import numpy as np

sys.path.insert(0, "/opt/trn_rl_repo")

import ml_dtypes  # noqa: E402

from concourse import bacc, mybir  # noqa: E402
import concourse.tile as tile  # noqa: E402
from concourse.bass_utils import run_bass_kernel_spmd  # noqa: E402

B, T, D, H, O, E = 4, 2048, 1024, 2048, 1024, 8
BT = B * T
N_CORES = 8
SHARD = BT // N_CORES  # 1024 output rows per core
P = 128
CAP = 2560          # per-expert token capacity (multiple of NCH)
KD = D // P         # 8 contraction tiles over D
MH = H // P         # 16 partition tiles over H
NCH = 512           # token chunk = one PSUM bank in fp32
NCHUNK = CAP // NCH  # 5
TT = NCH // P       # 4 token tiles per chunk
TTOT = CAP // P     # 20 token tiles total
OC = 512            # output free-dim chunk
NOC = O // OC       # 2
NIDX = CAP // 16    # scatter index columns

f32 = mybir.dt.float32
bf16 = mybir.dt.bfloat16
i16 = mybir.dt.int16
nbf16 = ml_dtypes.bfloat16
AF = mybir.ActivationFunctionType
ALU = mybir.AluOpType

_CACHE: dict = {}


def _build():
    nc = bacc.Bacc("TRN2", target_bir_lowering=False, debug=False,
                   num_devices=N_CORES)
    xs = nc.declare_dram_parameter("xs", [SHARD, D], bf16, isOutput=False)
    w1 = nc.declare_dram_parameter("w1", [D, H], bf16, isOutput=False)
    w2 = nc.declare_dram_parameter("w2", [H, O], bf16, isOutput=False)
    b1 = nc.declare_dram_parameter("b1", [H, 1], f32, isOutput=False)
    gt = nc.declare_dram_parameter("gt", [CAP, 1], f32, isOutput=False)
    idx = nc.declare_dram_parameter("idx", [P, NIDX], i16, isOutput=False)
    y = nc.declare_dram_parameter("y", [SHARD, O], bf16, isOutput=True)

    with tile.TileContext(nc) as tc:
        with (
            tc.tile_pool(name="const", bufs=1) as constp,
            tc.tile_pool(name="xres", bufs=1) as xp,
            tc.tile_pool(name="wres", bufs=1) as wp,
            tc.tile_pool(name="hbuf", bufs=2) as hp,
            tc.tile_pool(name="yall", bufs=1) as yp,
            tc.tile_pool(name="dram", bufs=1, space="DRAM") as dramp,
            tc.tile_pool(name="ps1", bufs=4, space="PSUM") as ps1,
            tc.tile_pool(name="ps2", bufs=4, space="PSUM") as ps2,
        ):
            # ---- constants: biases, gate weights, scatter indices ----
            b1sb = constp.tile([P, MH], f32, tag="b1sb")
            nc.sync.dma_start(
                out=b1sb[:],
                in_=b1.rearrange("(hm p) one -> p (hm one)", p=P))
            gtsb = constp.tile([P, TTOT], f32, tag="gtsb")
            nc.sync.dma_start(
                out=gtsb[:],
                in_=gt.rearrange("(tt p) one -> p (tt one)", p=P))
            idxsb = constp.tile([P, NIDX], i16, tag="idxsb")
            nc.sync.dma_start(out=idxsb[:], in_=idx[:, :])

            # ---- zero the local combine accumulator [BT, O] ----
            acc = dramp.tile([BT, O], bf16, tag="acc")
            zt = constp.tile([P, O], bf16, tag="zt")
            nc.gpsimd.memset(zt[:], 0.0)
            for r in range(BT // P):
                nc.sync.dma_start(out=acc[r * P:(r + 1) * P, :], in_=zt[:])

            # ---- x: AllGather token shards, gather+transpose own tokens ----
            inb = dramp.tile([SHARD, D], bf16, tag="inb")
            nc.sync.dma_start(out=inb[:], in_=xs[:, :])
            xfull = dramp.tile([BT, D], bf16, tag="xfull")
            nc.gpsimd.collective_compute(
                "AllGather",
                ALU.bypass,
                replica_groups=[list(range(N_CORES))],
                ins=[inb[:]],
                outs=[xfull[:]])
            w1sb = []
            for kd in range(KD):
                t = wp.tile([P, H], bf16, tag=f"w1_{kd}")
                nc.sync.dma_start(out=t[:], in_=w1[kd * P:(kd + 1) * P, :])
                w1sb.append(t)
            w2sb = []
            for kh in range(MH):
                t = wp.tile([P, O], bf16, tag=f"w2_{kh}")
                nc.sync.dma_start(out=t[:], in_=w2[kh * P:(kh + 1) * P, :])
                w2sb.append(t)

            # ---- FFN over token chunks; scaled outputs land in ysb_all ----
            ysb_all = yp.tile([P, TTOT * O], bf16, tag="ysb")
            for c in range(NCHUNK):
                # gather+transpose this chunk's tokens: [D-part, 512 tok]
                # (a single whole-CAP transposing gather crashes the DGE)
                xc = xp.tile([P, KD * NCH], bf16, tag="xc", bufs=2)
                nc.gpsimd.dma_gather(
                    xc[:].rearrange("p (kd t) -> p kd t", t=NCH),
                    xfull[:],
                    idxsb[:, c * (NCH // 16):(c + 1) * (NCH // 16)],
                    NCH, NCH, D, transpose=True)
                hts = []
                for hm in range(MH):
                    ph = ps1.tile([P, NCH], f32, tag="ph")
                    for kd in range(KD):
                        nc.tensor.matmul(
                            ph[:], lhsT=w1sb[kd][:, hm * P:(hm + 1) * P],
                            rhs=xc[:, kd * NCH:(kd + 1) * NCH],
                            start=(kd == 0), stop=(kd == KD - 1))
                    ht = hp.tile([P, NCH], bf16, tag=f"h{hm}")
                    nc.scalar.activation(out=ht[:], in_=ph[:], func=AF.Relu,
                                         bias=b1sb[:, hm:hm + 1])
                    hts.append(ht)
                for tt in range(TT):
                    tglob = c * TT + tt
                    for oc in range(NOC):
                        po = ps2.tile([P, OC], f32, tag="po")
                        for kh in range(MH):
                            nc.tensor.matmul(
                                po[:], lhsT=hts[kh][:, tt * P:(tt + 1) * P],
                                rhs=w2sb[kh][:, oc * OC:(oc + 1) * OC],
                                start=(kh == 0), stop=(kh == MH - 1))
                        nc.vector.tensor_tensor(
                            out=ysb_all[:, tglob * O + oc * OC:
                                        tglob * O + (oc + 1) * OC],
                            in0=po[:],
                            in1=gtsb[:, tglob:tglob + 1].to_broadcast([P, OC]),
                            op=ALU.mult)

            # ---- scatter into the accumulator, combine across cores ----
            nc.gpsimd.dma_scatter_add(
                acc[:],
                ysb_all[:].rearrange("p (t o) -> p t o", o=O),
                idxsb[:],
                CAP,
                CAP,
                O)
            rsout = dramp.tile([SHARD, O], bf16, tag="rsout")
            nc.gpsimd.collective_compute(
                "ReduceScatter",
                ALU.add,
                replica_groups=[list(range(N_CORES))],
                ins=[acc[:]],
                outs=[rsout[:]])
            nc.sync.dma_start(out=y[:, :], in_=rsout[:])

    nc.compile()
    return nc


def kernel(x, num_experts_chosen, W_gate, b_gate, W1, b1, W2, b2):
    assert int(num_experts_chosen) == 2
    x2d = np.asarray(x, np.float32).reshape(BT, D)
    Wg = np.asarray(W_gate, np.float64)
    bg = np.asarray(b_gate, np.float64)
    W1 = np.asarray(W1, np.float32)
    b1 = np.asarray(b1, np.float32)
    W2 = np.asarray(W2, np.float32)
    b2 = np.asarray(b2, np.float32)

    # ---- router on host: softmax over experts, top-2, L1 renormalize ----
    logits = x2d.astype(np.float64) @ Wg + bg
    order = np.argsort(-logits, axis=-1, kind="stable")  # ties: lower index
    top2 = order[:, :2]
    mx = logits.max(-1, keepdims=True)
    pexp = np.exp(logits - mx)
    gating = pexp / pexp.sum(-1, keepdims=True)
    pv = np.take_along_axis(gating, top2, 1)
    g = (pv / np.maximum(pv.sum(1, keepdims=True), 1e-12)).astype(np.float32)

    xbf = x2d.astype(nbf16)
    W1b = W1.astype(nbf16)
    W2b = W2.astype(nbf16)

    idx_e, g_e = [], []
    for e in range(E):
        s0 = top2[:, 0] == e
        s1 = top2[:, 1] == e
        sel = np.nonzero(s0 | s1)[0]
        ge = np.where(s0[sel], g[sel, 0], g[sel, 1]).astype(np.float32)
        idx_e.append(sel)
        g_e.append(ge)

    if "nc" not in _CACHE:
        _CACHE["nc"] = _build()
    nc = _CACHE["nc"]

    out2d = np.zeros((BT, O), np.float32)
    maxn = max(len(i) for i in idx_e)
    rounds = max(1, -(-maxn // CAP))
    for r in range(rounds):
        in_maps = []
        for e in range(E):
            sl = idx_e[e][r * CAP:(r + 1) * CAP]
            n = len(sl)
            gtb = np.zeros((CAP, 1), np.float32)
            dest = np.zeros(CAP, np.int16)  # pad -> row 0, zero gate
            if n:
                gtb[:n, 0] = g_e[e][r * CAP:r * CAP + n]
                dest[:n] = sl.astype(np.int16)
            # idx table is read per-16-partition group by the 8 gpsimd
            # cores -> must be replicated into all 8 groups
            idxb = np.tile(dest.reshape(NIDX, 16).T, (8, 1))
            in_maps.append({
                "xs": xbf[e * SHARD:(e + 1) * SHARD], "w1": W1b[e],
                "w2": W2b[e],
                "b1": np.ascontiguousarray(b1[e][:, None]), "gt": gtb,
                "idx": idxb,
            })
        res = run_bass_kernel_spmd(nc, in_maps, core_ids=list(range(N_CORES)))
        for c in range(N_CORES):
            out2d[c * SHARD:(c + 1) * SHARD] += \
                res.results[c]["y"].astype(np.float32)

    if b2.any():
        out2d += g[:, 0, None] * b2[top2[:, 0]] \
            + g[:, 1, None] * b2[top2[:, 1]]
    return out2d.reshape(B, T, O)
